# revision 13
# baseline (speedup 1.0000x reference)
"""Trainium2 Bass kernel for nn_DecoderRNN (attention LSTM decoder).

Strategy: pure data-parallel over batch (16 rows/core on 8 cores, no
collectives).  Everything on-device is feature-major ("transposed") so all
elementwise work runs on full 128-partition tiles; all matmuls are
weight-stationary with bf16 operands (fp32 PSUM accumulation, fp32 cell
state).  The per-timestep x-dependent contributions to the attention scores
and the LSTM gates are precomputed on-device for all timesteps in two big
matmuls; the vocab projection is interleaved after the recurrence in chunks.

sigmoid(x) is computed as 0.5 + 0.5*tanh(x/2) (the 0.5 folded into the
i/f/o weight rows on the host) so the whole kernel needs only the exp/tanh
ACT table set.
"""
import sys
import numpy as np

sys.path.insert(0, "/opt/trn_rl_repo")

import ml_dtypes

B, T, E, H, V, A = 128, 31, 512, 512, 10000, 2048
NCORES = 8
BL = B // NCORES          # 16 batch rows per core
NT = T + 1                # 32 timesteps incl. t=0
R = T * BL                # 496 gathered tokens per core
RPAD = 512                # padded so num_idxs % 128 == 0
G4 = 4 * H                # 2048 gate rows
VT = (V + 127) // 128     # 79 vocab tiles (last ragged: 16 rows)

BF16 = ml_dtypes.bfloat16
_BUILT = {}


def _bf(x):
    return np.ascontiguousarray(np.asarray(x, np.float32), dtype=BF16)


def _fmajor(x2d):
    """[128*ntile, cols] -> [128, ntile*cols], tile-major feature layout."""
    rows, cols = x2d.shape
    nt = rows // 128
    return np.ascontiguousarray(
        x2d.reshape(nt, 128, cols).transpose(1, 0, 2).reshape(128, nt * cols)
    )


def _build_program():
    """Build the Bass program (single SPMD program, 8 cores)."""
    import concourse.bass as bass
    import concourse.mybir as mybir
    from concourse import tile as tile_mod
    from concourse.tile import TileContext

    def _drain_and_barrier(self, tick_clock, wait_clock):
        drain_inst = self.nc.sync.drain()
        wait_clock.add_sem_waits(
            drain_inst.ins, tile_mod.ScopedClock({None: tick_clock.global_clock})
        )
        self.nc.all_engine_barrier()
        assert self.sems is not None
        popped = self.nc._tile_sem_poison_stack.pop()
        assert popped is self._sem_poison
        self.nc.clear_and_free_semaphores(list(self.sems.allocated().values()))
        self.nc.all_engine_barrier()

    TileContext._drain_and_barrier = _drain_and_barrier

    fp32 = mybir.dt.float32
    bf16 = mybir.dt.bfloat16
    i16 = mybir.dt.int16
    TANH = mybir.ActivationFunctionType.Tanh
    EXP = mybir.ActivationFunctionType.Exp
    MUL = mybir.AluOpType.mult
    ADD = mybir.AluOpType.add

    nc = bass.Bass("TRN2", target_bir_lowering=False)

    # ---- I/O (all weight tensors pre-tiled [128, ntile*cols] on host) ---
    d_xsT = nc.declare_dram_parameter("xsT", [128, 4, RPAD], bf16, isOutput=False)
    d_featT = nc.declare_dram_parameter("featT", [128, 4 * BL], bf16, isOutput=False)
    d_cnnT = nc.declare_dram_parameter("cnnT", [128, 16 * BL], bf16, isOutput=False)
    d_wanhT = nc.declare_dram_parameter("wanhT", [128, 4 * A], bf16, isOutput=False)
    d_wanxT = nc.declare_dram_parameter("wanxT", [128, 4 * A], bf16, isOutput=False)
    d_wadT = nc.declare_dram_parameter("wadT", [128, 16 * E], bf16, isOutput=False)
    d_mxT = nc.declare_dram_parameter("mxT", [128, 4 * G4], bf16, isOutput=False)
    d_wihT = nc.declare_dram_parameter("wihT", [128, 4 * G4], bf16, isOutput=False)
    d_whhT = nc.declare_dram_parameter("whhT", [128, 4 * G4], bf16, isOutput=False)
    d_woutT = nc.declare_dram_parameter("woutT", [128, 4, V], bf16, isOutput=False)
    d_battn = nc.declare_dram_parameter("battn", [1, A], bf16, isOutput=False)
    d_bg = nc.declare_dram_parameter("bg", [1, G4], bf16, isOutput=False)
    d_bg0 = nc.declare_dram_parameter("bg0", [1, G4], bf16, isOutput=False)
    d_ones = nc.declare_dram_parameter("onesrow", [1, RPAD], bf16, isOutput=False)
    d_onesf = nc.declare_dram_parameter("onesf", [1, 128], fp32, isOutput=False)
    d_onescol = nc.declare_dram_parameter("onescol", [128, 1], bf16, isOutput=False)
    d_logitsT = nc.declare_dram_parameter("logitsT", [V, NT * BL], fp32, isOutput=True)

    with TileContext(nc) as tc:
        with tc.tile_pool(name="persist", bufs=1) as pw, \
             tc.tile_pool(name="psums", bufs=2, space="PSUM") as pp:
            wanhT = pw.tile([128, 4, A], bf16)
            wadT = pw.tile([128, 16, E], bf16)
            wihT = pw.tile([128, 4, G4], bf16)
            whhT = pw.tile([128, 4, G4], bf16)
            featT = pw.tile([128, 4, BL], bf16)
            cnnT = pw.tile([128, 16, BL], bf16)
            battn = pw.tile([1, A], bf16)
            bg = pw.tile([1, G4], bf16)
            bg0 = pw.tile([1, G4], bf16)
            onesrow = pw.tile([1, RPAD], bf16)
            onesf = pw.tile([1, 128], fp32)
            onescol = pw.tile([128, 1], bf16)
            preA = pw.tile([128, 16, RPAD], bf16)
            preG = pw.tile([128, 16, RPAD], bf16)
            hidT = pw.tile([128, 4, NT * BL], bf16)
            c_sb = pw.tile([128, 4, BL], fp32)

            for dst, src in [
                (wanhT[:, :, :], d_wanhT), (wadT[:, :, :], d_wadT),
                (wihT[:, :, :], d_wihT), (whhT[:, :, :], d_whhT),
                (featT[:, :, :], d_featT), (cnnT[:, :, :], d_cnnT),
                (battn[:, :], d_battn), (bg[:, :], d_bg), (bg0[:, :], d_bg0),
                (onesrow[:, :], d_ones), (onesf[:, :], d_onesf),
                (onescol[:, :], d_onescol),
            ]:
                nc.sync.dma_start(out=dst, in_=src[:, :])

            # ---- phase 1+2: gather + precompute ------------------------
            with tc.tile_pool(name="pre", bufs=1) as ppre:
                wanxT = ppre.tile([128, 4, A], bf16)
                mxT = ppre.tile([128, 4, G4], bf16)
                xsT = ppre.tile([128, 4, RPAD], bf16)
                nc.sync.dma_start(out=wanxT[:, :, :], in_=d_wanxT[:, :])
                nc.sync.dma_start(out=mxT[:, :, :], in_=d_mxT[:, :])
                nc.sync.dma_start(out=xsT[:, :, :], in_=d_xsT[:, :, :])
                for (wsrc, brow, dstp) in ((wanxT, battn, preA), (mxT, bg, preG)):
                    for j in range(16):
                        ps = pp.tile([128, RPAD], fp32, tag="spsum")
                        for kk in range(4):
                            nc.tensor.matmul(
                                ps[:, :],
                                wsrc[:, kk, 128 * j:128 * j + 128],
                                xsT[:, kk, :],
                                start=(kk == 0), stop=False,
                            )
                        nc.tensor.matmul(
                            ps[:, :],
                            brow[0:1, 128 * j:128 * j + 128],
                            onesrow[0:1, :],
                            start=False, stop=True,
                        )
                        if j % 2 == 0:
                            nc.vector.tensor_copy(dstp[:, j, :], ps[:, :])
                        else:
                            nc.scalar.copy(dstp[:, j, :], ps[:, :])

            with tc.tile_pool(name="scratch", bufs=2) as sc:

                def lstm_tail(psum_g, pre_slice, t, first):
                    gat = sc.tile([128, 16, BL], fp32, tag="gat")
                    if pre_slice is not None:
                        pref = sc.tile([128, 16, BL], fp32, tag="pref")
                        nc.scalar.copy(pref[:, :, :], pre_slice)
                        nc.vector.tensor_add(gat[:, :, :], psum_g[:, :, :], pref[:, :, :])
                        src = gat
                    else:
                        src = psum_g
                    tg = sc.tile([128, 16, BL], fp32, tag="tg")
                    nc.scalar.activation(tg[:, :, :], src[:, :, :], TANH)
                    sif = sc.tile([128, 8, BL], fp32, tag="sif")
                    so = sc.tile([128, 4, BL], fp32, tag="so")
                    nc.vector.tensor_scalar(sif[:, :, :], tg[:, 0:8, :], 0.5, 0.5, MUL, ADD)
                    nc.vector.tensor_scalar(so[:, :, :], tg[:, 12:16, :], 0.5, 0.5, MUL, ADD)
                    ig = sc.tile([128, 4, BL], fp32, tag="ig")
                    nc.vector.tensor_mul(ig[:, :, :], sif[:, 0:4, :], tg[:, 8:12, :])
                    if first:
                        nc.vector.tensor_copy(c_sb[:, :, :], ig[:, :, :])
                    else:
                        fc = sc.tile([128, 4, BL], fp32, tag="fc")
                        nc.vector.tensor_mul(fc[:, :, :], sif[:, 4:8, :], c_sb[:, :, :])
                        nc.vector.tensor_add(c_sb[:, :, :], ig[:, :, :], fc[:, :, :])
                    tc2 = sc.tile([128, 4, BL], fp32, tag="tc2")
                    nc.scalar.activation(tc2[:, :, :], c_sb[:, :, :], TANH)
                    nc.vector.tensor_mul(
                        hidT[:, :, BL * t:BL * t + BL], so[:, :, :], tc2[:, :, :]
                    )

                # ---- step 0 --------------------------------------------
                ps_g = pp.tile([128, 16, BL], fp32, tag="gpsum")
                for j in range(16):
                    for kk in range(4):
                        nc.tensor.matmul(
                            ps_g[:, j, :],
                            wihT[:, kk, 128 * j:128 * j + 128],
                            featT[:, kk, :],
                            start=(kk == 0), stop=False,
                        )
                    nc.tensor.matmul(
                        ps_g[:, j, :],
                        bg0[0:1, 128 * j:128 * j + 128],
                        onesrow[0:1, 0:BL],
                        start=False, stop=True,
                    )
                lstm_tail(ps_g, None, 0, True)

                def proj_unit(g, c0, c1):
                    n = c1 - c0
                    v0 = 1024 * g
                    gw = min(1024, V - v0)
                    wsl = sc.tile([128, 4, 1024], bf16, tag="wosl", bufs=3)
                    nc.sync.dma_start(
                        out=wsl[:, :, 0:gw], in_=d_woutT[:, :, v0:v0 + gw]
                    )
                    for vv in range((gw + 127) // 128):
                        m = min(128, gw - 128 * vv)
                        pv = pp.tile([128, 128], fp32, tag="vpsum")
                        for kk in range(4):
                            nc.tensor.matmul(
                                pv[0:m, 0:n],
                                wsl[:, kk, 128 * vv:128 * vv + m],
                                hidT[:, kk, c0:c1],
                                start=(kk == 0), stop=(kk == 3),
                            )
                        st = sc.tile([128, 128], fp32, tag="vstage", bufs=6)
                        if vv % 2 == 0:
                            nc.vector.tensor_copy(st[0:m, 0:n], pv[0:m, 0:n])
                        else:
                            nc.scalar.copy(st[0:m, 0:n], pv[0:m, 0:n])
                        nc.sync.dma_start(
                            out=d_logitsT[v0 + 128 * vv:v0 + 128 * vv + m, c0:c1],
                            in_=st[0:m, 0:n],
                        )

                # chunk ci (hidden cols 128*ci..) is final after step 8*ci+7;
                # spread its 10 vocab groups over steps 8*ci+8 .. 8*ci+15.
                proj_sched = {}
                for ci in range(3):
                    for s in range(8):
                        t0 = 8 * ci + 8 + s
                        units = range(10 * s // 8, 10 * (s + 1) // 8)
                        proj_sched.setdefault(t0, []).extend(
                            (g, 128 * ci, 128 * ci + 128) for g in units
                        )

                # ---- recurrence t = 1..31 ------------------------------
                for t in range(1, NT):
                    rs = BL * (t - 1)

                    ps_s = pp.tile([128, 16, BL], fp32, tag="spsum")
                    for j in range(16):
                        for kk in range(4):
                            nc.tensor.matmul(
                                ps_s[:, j, :],
                                wanhT[:, kk, 128 * j:128 * j + 128],
                                hidT[:, kk, rs:rs + BL],
                                start=(kk == 0), stop=(kk == 3),
                            )
                    preAf = sc.tile([128, 16, BL], fp32, tag="preAf")
                    nc.scalar.copy(preAf[:, :, :], preA[:, :, rs:rs + BL])
                    sco = sc.tile([128, 16, BL], fp32, tag="sco")
                    nc.vector.tensor_add(sco[:, :, :], ps_s[:, :, :], preAf[:, :, :])
                    att = sc.tile([128, 16, BL], bf16, tag="att")
                    nc.scalar.activation(att[:, :, :], sco[:, :, :], EXP)
                    att2 = sc.tile([128, 16, BL], bf16, tag="att2")
                    nc.vector.tensor_mul(att2[:, :, :], att[:, :, :], cnnT[:, :, :])

                    ps_z = pp.tile([1, BL], fp32, tag="zrx")
                    for j in range(16):
                        nc.tensor.matmul(
                            ps_z[0:1, :],
                            onescol[:, 0:1],
                            att2[:, j, :],
                            start=(j == 0), stop=(j == 15),
                        )
                    rz = sc.tile([1, 4 * BL], fp32, tag="rz")
                    nc.vector.reciprocal(rz[0:1, 0:BL], ps_z[0:1, :])
                    for q in range(1, 4):
                        nc.vector.tensor_copy(rz[0:1, BL * q:BL * q + BL], rz[0:1, 0:BL])
                    ps_rz = pp.tile([128, 4, BL], fp32, tag="zrx")
                    nc.tensor.matmul(
                        ps_rz[:, :, :], onesf[0:1, :], rz[0:1, :],
                        start=True, stop=True,
                    )
                    rzbc = sc.tile([128, 4, BL], fp32, tag="rzbc")
                    nc.vector.tensor_copy(rzbc[:, :, :], ps_rz[:, :, :])

                    ps_x = pp.tile([128, 4, BL], fp32, tag="zrx")
                    for me in range(4):
                        for ka in range(16):
                            nc.tensor.matmul(
                                ps_x[:, me, :],
                                wadT[:, ka, 128 * me:128 * me + 128],
                                att2[:, ka, :],
                                start=(ka == 0), stop=(ka == 15),
                            )
                    x2aT = sc.tile([128, 4, BL], bf16, tag="x2aT")
                    nc.vector.tensor_mul(x2aT[:, :, :], ps_x[:, :, :], rzbc[:, :, :])

                    ps_g = pp.tile([128, 16, BL], fp32, tag="gpsum")
                    for j in range(16):
                        # W_hh half first: depends only on h(t-1), so the
                        # scheduler can issue it during the previous step's
                        # elementwise tail (keeps PE warm).
                        for kk in range(4):
                            nc.tensor.matmul(
                                ps_g[:, j, :],
                                whhT[:, kk, 128 * j:128 * j + 128],
                                hidT[:, kk, rs:rs + BL],
                                start=(kk == 0), stop=False,
                            )
                        for kk in range(4):
                            nc.tensor.matmul(
                                ps_g[:, j, :],
                                wihT[:, kk, 128 * j:128 * j + 128],
                                x2aT[:, kk, :],
                                start=False, stop=(kk == 3),
                            )
                    lstm_tail(ps_g, preG[:, :, rs:rs + BL], t, False)

                    for (g, c0, c1) in proj_sched.get(t, ()):
                        proj_unit(g, c0, c1)

                for g in range(10):
                    proj_unit(g, 384, 512)

    # post-pass: walrus in this container allows only 1 sem wait per
    # instruction; move extras onto same-engine NoOps inserted just before.
    nid = 0
    for f in nc.m.functions:
        for bb in f.blocks:
            insts = bb.instructions
            i = 0
            while i < len(insts):
                ins = insts[i]
                si = ins.sync_info
                if si is not None and len(si.on_wait) > 1:
                    waits = list(si.on_wait)
                    si.on_wait = waits[-1:]
                    for w in waits[:-1]:
                        nid += 1
                        nop = mybir.InstNoOp(
                            name=f"WS-{nid}",
                            sync_info=mybir.SyncInfo(on_wait=[w], on_update=[]),
                            bass_nofuse=True,
                            engine=ins.engine,
                        )
                        insts.insert(i, nop)
                        i += 1
                i += 1
    return nc


def _prep_inputs(inputs):
    f32 = np.float32
    features = np.asarray(inputs["features"], f32)
    cnn = np.asarray(inputs["cnn_features"], f32)
    captions = np.asarray(inputs["captions"])
    emb = np.asarray(inputs["embed_table"], f32)
    W_ih = np.asarray(inputs["W_ih"], f32)
    W_hh = np.asarray(inputs["W_hh"], f32)
    b_ih = np.asarray(inputs["b_ih"], f32)
    b_hh = np.asarray(inputs["b_hh"], f32)
    W_attn = np.asarray(inputs["W_attn"], f32)
    b_attn = np.asarray(inputs["b_attn"], f32)
    W_attd = np.asarray(inputs["W_attd"], f32)
    b_attd = np.asarray(inputs["b_attd"], f32)
    W_out = np.asarray(inputs["W_out"], f32)

    s = np.ones((G4, 1), f32)
    s[0:H] = 0.5
    s[H:2 * H] = 0.5
    s[3 * H:4 * H] = 0.5
    Mx = W_ih @ W_attd[:, :E]
    bias_g = (b_ih + b_hh + W_ih @ b_attd) * s[:, 0]
    bias_g0 = (b_ih + b_hh) * s[:, 0]

    common = {
        "wanhT": _fmajor(_bf(W_attn[:, E:].T)),
        "wanxT": _fmajor(_bf(W_attn[:, :E].T)),
        "wadT": _fmajor(_bf(W_attd[:, E:].T)),
        "mxT": _fmajor(_bf((Mx * s).T)),
        "wihT": _fmajor(_bf((W_ih * s).T)),
        "whhT": _fmajor(_bf((W_hh * s).T)),
        "woutT": _fmajor(_bf(W_out.T)).reshape(128, 4, V),
        "battn": _bf(b_attn[None, :]),
        "bg": _bf(bias_g[None, :]),
        "bg0": _bf(bias_g0[None, :]),
        "onesrow": _bf(np.ones((1, RPAD), f32)),
        "onesf": np.ones((1, 128), f32),
        "onescol": _bf(np.ones((128, 1), f32)),
    }
    in_maps = []
    for k in range(NCORES):
        bsl = slice(BL * k, BL * k + BL)
        toks = captions[bsl].astype(np.int64).T.reshape(-1)   # r=(t-1)*16+b
        xs = np.zeros((RPAD, E), np.float32)
        xs[:R] = emb[toks]
        in_maps.append({
            **common,
            "xsT": _fmajor(_bf(xs.T)).reshape(128, 4, RPAD),
            "featT": _fmajor(_bf(features[bsl].T)),
            "cnnT": _fmajor(_bf(cnn[bsl].T)),
        })
    return in_maps


def kernel(**inputs):
    from concourse.bass_utils import run_bass_kernel_spmd

    if "nc" not in _BUILT:
        _BUILT["nc"] = _build_program()
    nc = _BUILT["nc"]
    in_maps = _prep_inputs(inputs)
    res = run_bass_kernel_spmd(nc, in_maps, list(range(NCORES)))

    b_out = np.asarray(inputs["b_out"], np.float32)
    out = np.empty((NT * B, V), np.float32)
    o3 = out.reshape(NT, B, V)
    for k in range(NCORES):
        lt = res.results[k]["logitsT"]                        # [V, 32*16]
        o3[:, BL * k:BL * k + BL, :] = lt.reshape(V, NT, BL).transpose(1, 2, 0)
    out += b_out[None, :]
    return out
